# revision 11
# baseline (speedup 1.0000x reference)
"""Trainium2 Bass kernel for nn_DIoUAnswerSpanLoss.

Algorithm
---------
The reference builds a [B, L, L] score matrix score[b,i,j] = ls[b,i] + le[b,j]
(log-softmaxed logits), masks the lower triangle and pad positions, takes
argmax over (i, j), then computes a DIoU-style loss from the resulting integer
span positions (and the ground-truth spans).

Two exact reductions make this cheap:
  1. log_softmax subtracts a per-row constant, which never changes any
     argmax, so raw logits can be used directly for position finding.
  2. max_i<=j (msl[i] + el[j]) = cummax(msl)[j] + el[j], so the [L, L]
     matrix never needs materializing - a prefix max suffices.

Per row: msl = mask ? sl : -BIG; m = cummax(msl); col = mask ? m + el : -BIG;
j* = first argmax(col); v* = m[j*]; i* = first index with m == v*.

Distribution: the whole computation per core is ~15us of straight-line work,
while an 8-core AllReduce of even 16 bytes costs ~45us of latency on this
fabric. So instead of sharding the batch and reducing, every core receives
the full input and computes the full scalar loss independently - zero
communication - and the answer is read from core 0.

Layout: each of the 32 rows occupies 4 SBUF partitions x 512 elements
(p = 4*row + seg, global index g = 512*p + f - 65536, kept negative so a
zero from a non-matching lane never wins a reduce_min). The prefix max runs
as a segmented tensor_tensor_scan; segment boundaries are stitched with a
second scan over the PE-transposed segment ends (a -1e30 additive "reset" at
each row start keeps rows independent). Per-partition (max, first-argmax)
come from the native MAX8/FIND_INDEX8 pair. Cross-partition moves use fp32
matmuls with one-hot constants (exact: every product is x*1.0 or x*0.0);
the per-row selection then runs in a [32 rows x 4 seg] domain where the row
reductions are plain per-partition reduces and every broadcast is a
per-partition tensor_scalar operand. First-occurrence argmax semantics
(matching jnp.argmax) fall out of reduce_min over (value==max)*(negative
global index). The global-index offset cancels in every DIoU term after a
single re-basing subtract, so ground-truth positions are passed pre-offset.
"""

import numpy as np

import concourse.bass as bass
import concourse.bacc as bacc
import concourse.mybir as mybir
from concourse import tile
from concourse.bass_utils import run_bass_kernel_spmd

B, L = 32, 2048
NCORES = 8
SEG = 4                    # segments (partitions) per row
FREE = L // SEG            # 512 elements per segment
P = B * SEG                # 128 partitions
NEGBIG = -1.0e30
GOFF = float(P * FREE)     # 65536: global index offset
OFF = 4096.0               # offset used for the DIoU position space

F32 = mybir.dt.float32
U16 = mybir.dt.uint16
A = mybir.AluOpType
AX = mybir.AxisListType.X

_CACHE = {}


def _tt(nc, out, a, b, op):
    nc.vector.tensor_tensor(out, a, b, op)


def _build_program():
    nc = bacc.Bacc("TRN2", target_bir_lowering=False, debug=False, num_devices=NCORES)

    sl = nc.dram_tensor("sl", [P, FREE], F32, kind="ExternalInput")
    el = nc.dram_tensor("el", [P, FREE], F32, kind="ExternalInput")
    cid = nc.dram_tensor("cid", [P, FREE], mybir.dt.int16, kind="ExternalInput")
    # constants packed into three shape-compatible bundles (fewer DMAs)
    pk1 = nc.dram_tensor("pk1", [P, P + FREE + SEG + B], F32, kind="ExternalInput")
    pk2 = nc.dram_tensor("pk2", [B, P + 5], F32, kind="ExternalInput")
    pk3 = nc.dram_tensor("pk3", [1, P + 1], F32, kind="ExternalInput")
    loss = nc.dram_tensor("loss", [1], F32, kind="ExternalOutput")

    with tile.TileContext(nc) as tc:
        with (
            tc.tile_pool(name="sb", bufs=1) as sb,
            tc.tile_pool(name="ps", bufs=4, space="PSUM") as ps,
        ):
            # ---- loads (critical ones first) ----
            sl_s = sb.tile([P, FREE], F32)
            cid_s = sb.tile([P, FREE], mybir.dt.int16)
            el_s = sb.tile([P, FREE], F32)
            pk1_s = sb.tile([P, P + FREE + SEG + B], F32)
            pk2_s = sb.tile([B, P + 5], F32)
            pk3_s = sb.tile([1, P + 1], F32)
            nc.sync.dma_start(cid_s[:], cid[:])
            nc.sync.dma_start(sl_s[:], sl[:])
            nc.sync.dma_start(pk3_s[:], pk3[:])
            nc.sync.dma_start(pk1_s[:], pk1[:])
            nc.sync.dma_start(el_s[:], el[:])
            nc.sync.dma_start(pk2_s[:], pk2[:])
            id128_s = pk1_s[:, 0:P]
            iotag_s = pk1_s[:, P : P + FREE]
            sel4_s = pk1_s[:, P + FREE : P + FREE + SEG]
            ematt_s = pk1_s[:, P + FREE + SEG : P + FREE + SEG + B]
            emat_s = pk2_s[:, 0:P]
            ones32_s = pk2_s[:, P : P + 1]
            gt32_s = pk2_s[:, P + 1 : P + 3]
            rb32_s = pk2_s[:, P + 3 : P + 5]
            resetv_s = pk3_s[:, 0:P]
            ones1_s = pk3_s[:, P : P + 1]

            # ---- masked start logits ----
            penalty = sb.tile([P, FREE], F32)
            nc.vector.tensor_scalar(
                penalty[:], cid_s[:], 0.0, NEGBIG, A.is_equal, A.mult
            )
            msl = sb.tile([P, FREE], F32)
            _tt(nc, msl[:], sl_s[:], penalty[:], A.add)

            # ---- segmented prefix max (within each 512-elem segment) ----
            mseg = sb.tile([P, FREE], F32)
            nc.vector.tensor_tensor_scan(
                mseg[:], msl[:], msl[:], NEGBIG, A.max, A.max
            )

            # ---- stitch segments: exclusive cross-segment prefix max ----
            psA = ps.tile([1, P], F32, tag="ps")
            nc.tensor.matmul(psA[:], mseg[:, FREE - 1 : FREE], id128_s)
            inclT = sb.tile([1, P], F32)
            nc.vector.tensor_tensor_scan(
                inclT[:], resetv_s, psA[:], NEGBIG, A.add, A.max
            )
            exclT = sb.tile([1, P], F32)
            nc.vector.memset(exclT[0:1, 0:1], NEGBIG)
            _tt(nc, exclT[0:1, 1:P], inclT[0:1, 0 : P - 1], pk3_s[0:1, 1:P], A.add)
            psE = ps.tile([P, 1], F32, tag="ps")
            nc.tensor.matmul(psE[:], exclT[:], ones1_s)
            m = sb.tile([P, FREE], F32)
            nc.vector.tensor_scalar(m[:], mseg[:], psE[:], None, A.max)

            # ---- column scores: col = m + el + penalty ----
            mel = sb.tile([P, FREE], F32)
            _tt(nc, mel[:], el_s[:], penalty[:], A.add)
            col = sb.tile([P, FREE], F32)
            _tt(nc, col[:], m[:], mel[:], A.add)

            # ---- per-partition top-1 value + first index of col ----
            top8 = sb.tile([P, 8], F32)
            nc.vector.max(top8[:], col[:])
            pm = top8[:, 0:1]
            idx8 = sb.tile([P, 8], U16)
            nc.vector.max_index(idx8[:], top8[:], col[:])
            idxf = sb.tile([P, 1], F32)
            nc.vector.tensor_copy(idxf[:], idx8[:, 0:1])
            pjt = sb.tile([P, 1], F32)
            pj = pjt[:]
            nc.vector.tensor_scalar(pj, idxf[:], pk1_s[:, P : P + 1], None, A.add)

            # ---- per-partition m value at that index (exact gather) ----
            candv = sb.tile([P, FREE], F32)
            nc.vector.scalar_tensor_tensor(
                candv[:], iotag_s, pj, m[:], A.is_equal, A.mult
            )
            pvt = sb.tile([P, 1], F32)
            pv = pvt[:]
            nc.vector.reduce_sum(pv, candv[:], axis=AX)

            # ---- fold [128,1] vectors into [32 rows x 4 seg] ----
            rhs12 = sb.tile([P, 3 * SEG], F32)
            nc.vector.tensor_scalar(rhs12[:, 0:SEG], sel4_s, pm, None, A.mult)
            nc.vector.tensor_scalar(
                rhs12[:, SEG : 2 * SEG], sel4_s, pj, None, A.mult
            )
            nc.vector.tensor_scalar(
                rhs12[:, 2 * SEG : 3 * SEG], sel4_s, pv, None, A.mult
            )
            psT = ps.tile([B, 3 * SEG], F32, tag="ps")
            nc.tensor.matmul(psT[:], ematt_s, rhs12[:])
            sT = sb.tile([B, 3 * SEG], F32)
            nc.vector.tensor_copy(sT[:], psT[:])
            pmr = sT[:, 0:SEG]
            pjr = sT[:, SEG : 2 * SEG]
            pvr = sT[:, 2 * SEG : 3 * SEG]

            # ---- per-row selection, all per-partition now ----
            m4 = sb.tile([B, 1], F32)
            nc.vector.reduce_max(m4[:], pmr, axis=AX)
            c1 = sb.tile([B, SEG], F32)
            nc.vector.tensor_scalar(c1[:], pmr, m4[:], None, A.is_equal)
            cj = sb.tile([B, SEG], F32)
            _tt(nc, cj[:], c1[:], pjr, A.mult)
            pos32 = sb.tile([B, 2], F32)
            j4 = pos32[:, 1:2]
            nc.vector.tensor_reduce(j4, cj[:], axis=AX, op=A.min)
            c2 = sb.tile([B, SEG], F32)
            nc.vector.tensor_scalar(c2[:], cj[:], j4, None, A.is_equal)
            cv = sb.tile([B, SEG], F32)
            _tt(nc, cv[:], c2[:], pvr, A.mult)
            v4 = sb.tile([B, 1], F32)
            nc.vector.reduce_sum(v4[:], cv[:], axis=AX)

            # ---- broadcast v* back per partition, find i* ----
            psVb = ps.tile([P, 1], F32, tag="ps")
            nc.tensor.matmul(psVb[:], emat_s, v4[:])
            candi = sb.tile([P, FREE], F32)
            nc.vector.scalar_tensor_tensor(
                candi[:], m[:], psVb[:], iotag_s, A.is_equal, A.mult
            )
            pit = sb.tile([P, 1], F32)
            pi = pit[:]
            nc.vector.tensor_reduce(pi, candi[:], axis=AX, op=A.min)
            rhs4 = sb.tile([P, SEG], F32)
            nc.vector.tensor_scalar(rhs4[:], sel4_s, pi, None, A.mult)
            psI = ps.tile([B, SEG], F32, tag="ps")
            nc.tensor.matmul(psI[:], ematt_s, rhs4[:])
            i4 = pos32[:, 0:1]
            nc.vector.tensor_reduce(i4, psI[:], axis=AX, op=A.min)

            # ---- DIoU from positions, [32 x *] domain ----
            posn = sb.tile([B, 2], F32)     # i*-4096 || j*-4096 per row
            _tt(nc, posn[:], pos32[:], rb32_s, A.subtract)
            ct2 = sb.tile([B, 2], F32)      # te = ep-sp || tg = gep-gsp
            _tt(nc, ct2[:, 0:1], posn[:, 1:2], posn[:, 0:1], A.subtract)
            _tt(nc, ct2[:, 1:2], gt32_s[:, 1:2], gt32_s[:, 0:1], A.subtract)
            cdg = sb.tile([B, 2], F32)      # cd || gcd = (x+1)*0.5
            nc.vector.tensor_scalar(cdg[:], ct2[:], 1.0, 0.5, A.add, A.mult)
            mnx = sb.tile([B, 2], F32)      # min(sp,gsp) || min(ep,gep)
            _tt(nc, mnx[:], posn[:], gt32_s, A.min)
            mxx = sb.tile([B, 2], F32)      # max(sp,gsp) || max(ep,gep)
            _tt(nc, mxx[:], posn[:], gt32_s, A.max)
            dd = sb.tile([B, 2], F32)       # cd-gcd || max_end-min_start
            _tt(nc, dd[:, 0:1], cdg[:, 0:1], cdg[:, 1:2], A.subtract)
            _tt(nc, dd[:, 1:2], mxx[:, 1:2], mnx[:, 0:1], A.subtract)
            sq3 = sb.tile([B, 3], F32)      # d1^2 || d2^2 || te+tg
            _tt(nc, sq3[:, 0:2], dd[:], dd[:], A.mult)
            _tt(nc, sq3[:, 2:3], ct2[:, 0:1], ct2[:, 1:2], A.add)
            dI = sb.tile([1, 1], F32)       # I = min(ep,gep)[0] - max(sp,gsp)[0]
            _tt(nc, dI[:], mnx[0:1, 1:2], mxx[0:1, 0:1], A.subtract)
            psS = ps.tile([1, 3], F32, tag="ps")
            nc.tensor.matmul(psS[:], ones32_s, sq3[:])

            # loss = 1 - I/(s3 - 32*I) + s1/s2
            u = sb.tile([1, 1], F32)
            nc.vector.scalar_tensor_tensor(
                u[:], dI[:], -float(B), psS[0:1, 2:3], A.mult, A.add
            )
            ru = sb.tile([1, 1], F32)
            nc.vector.reciprocal(ru[:], u[:])
            iou = sb.tile([1, 1], F32)
            _tt(nc, iou[:], dI[:], ru[:], A.mult)
            r2 = sb.tile([1, 1], F32)
            nc.vector.reciprocal(r2[:], psS[0:1, 1:2])
            cl = sb.tile([1, 1], F32)
            _tt(nc, cl[:], psS[0:1, 0:1], r2[:], A.mult)
            tmp2 = sb.tile([1, 1], F32)
            nc.vector.scalar_tensor_tensor(
                tmp2[:], iou[:], -1.0, cl[:], A.mult, A.add
            )
            lossv = sb.tile([1, 1], F32)
            nc.vector.tensor_scalar(lossv[:], tmp2[:], 1.0, None, A.add)
            nc.sync.dma_start(loss[:], lossv[:])

    nc.compile()
    return nc


def _constants():
    ones1 = np.ones((1, 1), dtype=np.float32)
    ones32 = np.ones((B, 1), dtype=np.float32)
    emat = np.zeros((B, P), dtype=np.float32)
    for k in range(B):
        emat[k, k * SEG : (k + 1) * SEG] = 1.0
    ematt = np.ascontiguousarray(emat.T)
    sel4 = (np.arange(P)[:, None] % SEG == np.arange(SEG)[None, :]).astype(np.float32)
    resetv = np.zeros((1, P), dtype=np.float32)
    resetv[0, ::SEG] = NEGBIG
    id128 = np.eye(P, dtype=np.float32)
    iotag = (
        float(FREE) * np.arange(P, dtype=np.float32)[:, None]
        + np.arange(FREE, dtype=np.float32)[None, :]
        - GOFF
    ).astype(np.float32)
    # global row base in (idx - 65536) space relative to the (idx - 4096) space
    rb32 = np.repeat(
        (2048.0 * np.arange(B, dtype=np.float32) - (GOFF - OFF))[:, None], 2, axis=1
    ).astype(np.float32)
    pk1 = np.concatenate([id128, iotag, sel4, ematt], axis=1).astype(np.float32)
    pk3 = np.concatenate([resetv, ones1], axis=1).astype(np.float32)
    return {"pk1": pk1, "pk3": pk3, "emat": emat, "ones32": ones32, "rb32": rb32}


def build_in_maps(c_ids, gt_start_positions, gt_end_positions, start_logits, end_logits):
    consts = _constants()
    cidf = np.ascontiguousarray(np.asarray(c_ids), dtype=np.int16).reshape(P, FREE)
    slf = np.ascontiguousarray(np.asarray(start_logits), dtype=np.float32).reshape(P, FREE)
    elf = np.ascontiguousarray(np.asarray(end_logits), dtype=np.float32).reshape(P, FREE)
    gt32 = np.stack(
        [
            np.asarray(gt_start_positions).astype(np.float32) - np.float32(OFF),
            np.asarray(gt_end_positions).astype(np.float32) - np.float32(OFF),
        ],
        axis=1,
    ).astype(np.float32)
    pk2 = np.concatenate(
        [consts["emat"], consts["ones32"], gt32, consts["rb32"]], axis=1
    ).astype(np.float32)
    core_map = {
        "sl": slf, "el": elf, "cid": cidf,
        "pk1": consts["pk1"], "pk2": pk2, "pk3": consts["pk3"],
    }
    return [dict(core_map) for _ in range(NCORES)]


def kernel(c_ids, gt_start_positions, gt_end_positions, start_logits, end_logits):
    if "nc" not in _CACHE:
        _CACHE["nc"] = _build_program()
    nc = _CACHE["nc"]
    in_maps = build_in_maps(
        c_ids, gt_start_positions, gt_end_positions, start_logits, end_logits
    )
    res = run_bass_kernel_spmd(nc, in_maps, core_ids=list(range(NCORES)))
    return np.asarray(res.results[0]["loss"], dtype=np.float32).reshape(())


# revision 12
# speedup vs baseline: 1.0410x; 1.0410x over previous
"""Trainium2 Bass kernel for nn_DIoUAnswerSpanLoss.

Algorithm
---------
The reference builds a [B, L, L] score matrix score[b,i,j] = ls[b,i] + le[b,j]
(log-softmaxed logits), masks the lower triangle and pad positions, takes
argmax over (i, j), then computes a DIoU-style loss from the resulting integer
span positions (and the ground-truth spans).

Two exact reductions make this cheap:
  1. log_softmax subtracts a per-row constant, which never changes any
     argmax, so raw logits can be used directly for position finding.
  2. max_i<=j (msl[i] + el[j]) = cummax(msl)[j] + el[j], so the [L, L]
     matrix never needs materializing - a prefix max suffices.

Per row: msl = mask ? sl : -BIG; m = cummax(msl); col = mask ? m + el : -BIG;
j* = first argmax(col); v* = m[j*]; i* = first index with m == v*.

Distribution: the whole computation per core is ~15us of straight-line work,
while an 8-core AllReduce of even 16 bytes costs ~45us of latency on this
fabric. So instead of sharding the batch and reducing, every core receives
the full input and computes the full scalar loss independently - zero
communication - and the answer is read from core 0.

Layout: each of the 32 rows occupies 4 SBUF partitions x 512 elements
(p = 4*row + seg, global index g = 512*p + f - 65536, kept negative so a
zero from a non-matching lane never wins a reduce_min). The prefix max runs
as a segmented tensor_tensor_scan; segment boundaries are stitched with a
second scan over the PE-transposed segment ends (a -1e30 additive "reset" at
each row start keeps rows independent). Per-partition (max, first-argmax)
come from the native MAX8/FIND_INDEX8 pair. Cross-partition moves use fp32
matmuls with one-hot constants (exact: every product is x*1.0 or x*0.0);
the per-row selection then runs in a [32 rows x 4 seg] domain where the row
reductions are plain per-partition reduces and every broadcast is a
per-partition tensor_scalar operand. First-occurrence argmax semantics
(matching jnp.argmax) fall out of reduce_min over (value==max)*(negative
global index). The global-index offset cancels in every DIoU term after a
single re-basing subtract, so ground-truth positions are passed pre-offset.
"""

import numpy as np

import concourse.bass as bass
import concourse.bacc as bacc
import concourse.mybir as mybir
from concourse import tile
from concourse.bass_utils import run_bass_kernel_spmd

B, L = 32, 2048
NCORES = 8
SEG = 4                    # segments (partitions) per row
FREE = L // SEG            # 512 elements per segment
P = B * SEG                # 128 partitions
NEGBIG = -1.0e30
GOFF = float(P * FREE)     # 65536: global index offset
OFF = 4096.0               # offset used for the DIoU position space

F32 = mybir.dt.float32
U16 = mybir.dt.uint16
A = mybir.AluOpType
AX = mybir.AxisListType.X

_CACHE = {}


def _tt(nc, out, a, b, op):
    nc.vector.tensor_tensor(out, a, b, op)


def _build_program():
    nc = bacc.Bacc("TRN2", target_bir_lowering=False, debug=False, num_devices=NCORES)

    sl = nc.dram_tensor("sl", [P, FREE], F32, kind="ExternalInput")
    el = nc.dram_tensor("el", [P, FREE], F32, kind="ExternalInput")
    cid = nc.dram_tensor("cid", [P, FREE], mybir.dt.int16, kind="ExternalInput")
    # constants packed into three shape-compatible bundles (fewer DMAs)
    id128 = nc.dram_tensor("id128", [P, P], F32, kind="ExternalInput")
    iotag = nc.dram_tensor("iotag", [P, FREE], F32, kind="ExternalInput")
    pk1 = nc.dram_tensor("pk1", [P, SEG + B], F32, kind="ExternalInput")
    pk2 = nc.dram_tensor("pk2", [B, P + 5], F32, kind="ExternalInput")
    pk3 = nc.dram_tensor("pk3", [1, P + 1], F32, kind="ExternalInput")
    loss = nc.dram_tensor("loss", [1], F32, kind="ExternalOutput")

    with tile.TileContext(nc) as tc:
        with (
            tc.tile_pool(name="sb", bufs=1) as sb,
            tc.tile_pool(name="ps", bufs=4, space="PSUM") as ps,
        ):
            # ---- loads (critical ones first) ----
            sl_s = sb.tile([P, FREE], F32)
            cid_s = sb.tile([P, FREE], mybir.dt.int16)
            el_s = sb.tile([P, FREE], F32)
            id128_t = sb.tile([P, P], F32)
            iotag_t = sb.tile([P, FREE], F32)
            pk1_s = sb.tile([P, SEG + B], F32)
            pk2_s = sb.tile([B, P + 5], F32)
            pk3_s = sb.tile([1, P + 1], F32)
            nc.sync.dma_start(cid_s[:], cid[:])
            nc.sync.dma_start(sl_s[:], sl[:])
            nc.sync.dma_start(pk3_s[:], pk3[:])
            nc.sync.dma_start(id128_t[:], id128[:])
            nc.sync.dma_start(el_s[:], el[:])
            nc.sync.dma_start(iotag_t[:], iotag[:])
            nc.sync.dma_start(pk1_s[:], pk1[:])
            nc.sync.dma_start(pk2_s[:], pk2[:])
            id128_s = id128_t[:]
            iotag_s = iotag_t[:]
            sel4_s = pk1_s[:, 0:SEG]
            ematt_s = pk1_s[:, SEG : SEG + B]
            emat_s = pk2_s[:, 0:P]
            ones32_s = pk2_s[:, P : P + 1]
            gt32_s = pk2_s[:, P + 1 : P + 3]
            rb32_s = pk2_s[:, P + 3 : P + 5]
            resetv_s = pk3_s[:, 0:P]
            ones1_s = pk3_s[:, P : P + 1]

            # ---- masked start logits ----
            penalty = sb.tile([P, FREE], F32)
            nc.vector.tensor_scalar(
                penalty[:], cid_s[:], 0.0, NEGBIG, A.is_equal, A.mult
            )
            msl = sb.tile([P, FREE], F32)
            _tt(nc, msl[:], sl_s[:], penalty[:], A.add)

            # ---- segmented prefix max (within each 512-elem segment) ----
            mseg = sb.tile([P, FREE], F32)
            nc.vector.tensor_tensor_scan(
                mseg[:], msl[:], msl[:], NEGBIG, A.max, A.max
            )

            # ---- stitch segments: exclusive cross-segment prefix max ----
            psA = ps.tile([1, P], F32, tag="ps")
            nc.tensor.matmul(psA[:], mseg[:, FREE - 1 : FREE], id128_s)
            inclT = sb.tile([1, P], F32)
            nc.vector.tensor_tensor_scan(
                inclT[:], resetv_s, psA[:], NEGBIG, A.add, A.max
            )
            exclT = sb.tile([1, P], F32)
            nc.vector.memset(exclT[0:1, 0:1], NEGBIG)
            _tt(nc, exclT[0:1, 1:P], inclT[0:1, 0 : P - 1], pk3_s[0:1, 1:P], A.add)
            psE = ps.tile([P, 1], F32, tag="ps")
            nc.tensor.matmul(psE[:], exclT[:], ones1_s)
            m = sb.tile([P, FREE], F32)
            nc.vector.tensor_scalar(m[:], mseg[:], psE[:], None, A.max)

            # ---- column scores: col = m + el + penalty ----
            mel = sb.tile([P, FREE], F32)
            _tt(nc, mel[:], el_s[:], penalty[:], A.add)
            col = sb.tile([P, FREE], F32)
            _tt(nc, col[:], m[:], mel[:], A.add)

            # ---- per-partition top-1 value + first index of col ----
            top8 = sb.tile([P, 8], F32)
            nc.vector.max(top8[:], col[:])
            pm = top8[:, 0:1]
            idx8 = sb.tile([P, 8], U16)
            nc.vector.max_index(idx8[:], top8[:], col[:])
            idxf = sb.tile([P, 1], F32)
            nc.vector.tensor_copy(idxf[:], idx8[:, 0:1])
            pjt = sb.tile([P, 1], F32)
            pj = pjt[:]
            nc.vector.tensor_scalar(pj, idxf[:], iotag_t[:, 0:1], None, A.add)

            # ---- per-partition m value at that index (exact gather) ----
            candv = sb.tile([P, FREE], F32)
            nc.vector.scalar_tensor_tensor(
                candv[:], iotag_s, pj, m[:], A.is_equal, A.mult
            )
            pvt = sb.tile([P, 1], F32)
            pv = pvt[:]
            nc.vector.reduce_sum(pv, candv[:], axis=AX)

            # ---- fold [128,1] vectors into [32 rows x 4 seg] ----
            rhs12 = sb.tile([P, 3 * SEG], F32)
            nc.vector.tensor_scalar(rhs12[:, 0:SEG], sel4_s, pm, None, A.mult)
            nc.vector.tensor_scalar(
                rhs12[:, SEG : 2 * SEG], sel4_s, pj, None, A.mult
            )
            nc.vector.tensor_scalar(
                rhs12[:, 2 * SEG : 3 * SEG], sel4_s, pv, None, A.mult
            )
            psT = ps.tile([B, 3 * SEG], F32, tag="ps")
            nc.tensor.matmul(psT[:], ematt_s, rhs12[:])
            sT = sb.tile([B, 3 * SEG], F32)
            nc.vector.tensor_copy(sT[:], psT[:])
            pmr = sT[:, 0:SEG]
            pjr = sT[:, SEG : 2 * SEG]
            pvr = sT[:, 2 * SEG : 3 * SEG]

            # ---- per-row selection, all per-partition now ----
            m4 = sb.tile([B, 1], F32)
            nc.vector.reduce_max(m4[:], pmr, axis=AX)
            c1 = sb.tile([B, SEG], F32)
            nc.vector.tensor_scalar(c1[:], pmr, m4[:], None, A.is_equal)
            cj = sb.tile([B, SEG], F32)
            _tt(nc, cj[:], c1[:], pjr, A.mult)
            pos32 = sb.tile([B, 2], F32)
            j4 = pos32[:, 1:2]
            nc.vector.tensor_reduce(j4, cj[:], axis=AX, op=A.min)
            c2 = sb.tile([B, SEG], F32)
            nc.vector.tensor_scalar(c2[:], cj[:], j4, None, A.is_equal)
            cv = sb.tile([B, SEG], F32)
            _tt(nc, cv[:], c2[:], pvr, A.mult)
            v4 = sb.tile([B, 1], F32)
            nc.vector.reduce_sum(v4[:], cv[:], axis=AX)

            # ---- broadcast v* back per partition, find i* ----
            psVb = ps.tile([P, 1], F32, tag="ps")
            nc.tensor.matmul(psVb[:], emat_s, v4[:])
            candi = sb.tile([P, FREE], F32)
            nc.vector.scalar_tensor_tensor(
                candi[:], m[:], psVb[:], iotag_s, A.is_equal, A.mult
            )
            pit = sb.tile([P, 1], F32)
            pi = pit[:]
            nc.vector.tensor_reduce(pi, candi[:], axis=AX, op=A.min)
            rhs4 = sb.tile([P, SEG], F32)
            nc.vector.tensor_scalar(rhs4[:], sel4_s, pi, None, A.mult)
            psI = ps.tile([B, SEG], F32, tag="ps")
            nc.tensor.matmul(psI[:], ematt_s, rhs4[:])
            i4 = pos32[:, 0:1]
            nc.vector.tensor_reduce(i4, psI[:], axis=AX, op=A.min)

            # ---- DIoU from positions, [32 x *] domain ----
            posn = sb.tile([B, 2], F32)     # i*-4096 || j*-4096 per row
            _tt(nc, posn[:], pos32[:], rb32_s, A.subtract)
            ct2 = sb.tile([B, 2], F32)      # te = ep-sp || tg = gep-gsp
            _tt(nc, ct2[:, 0:1], posn[:, 1:2], posn[:, 0:1], A.subtract)
            _tt(nc, ct2[:, 1:2], gt32_s[:, 1:2], gt32_s[:, 0:1], A.subtract)
            cdg = sb.tile([B, 2], F32)      # cd || gcd = (x+1)*0.5
            nc.vector.tensor_scalar(cdg[:], ct2[:], 1.0, 0.5, A.add, A.mult)
            mnx = sb.tile([B, 2], F32)      # min(sp,gsp) || min(ep,gep)
            _tt(nc, mnx[:], posn[:], gt32_s, A.min)
            mxx = sb.tile([B, 2], F32)      # max(sp,gsp) || max(ep,gep)
            _tt(nc, mxx[:], posn[:], gt32_s, A.max)
            dd = sb.tile([B, 2], F32)       # cd-gcd || max_end-min_start
            _tt(nc, dd[:, 0:1], cdg[:, 0:1], cdg[:, 1:2], A.subtract)
            _tt(nc, dd[:, 1:2], mxx[:, 1:2], mnx[:, 0:1], A.subtract)
            sq3 = sb.tile([B, 3], F32)      # d1^2 || d2^2 || te+tg
            _tt(nc, sq3[:, 0:2], dd[:], dd[:], A.mult)
            _tt(nc, sq3[:, 2:3], ct2[:, 0:1], ct2[:, 1:2], A.add)
            dI = sb.tile([1, 1], F32)       # I = min(ep,gep)[0] - max(sp,gsp)[0]
            _tt(nc, dI[:], mnx[0:1, 1:2], mxx[0:1, 0:1], A.subtract)
            psS = ps.tile([1, 3], F32, tag="ps")
            nc.tensor.matmul(psS[:], ones32_s, sq3[:])

            # loss = 1 - I/(s3 - 32*I) + s1/s2
            u = sb.tile([1, 1], F32)
            nc.vector.scalar_tensor_tensor(
                u[:], dI[:], -float(B), psS[0:1, 2:3], A.mult, A.add
            )
            ru = sb.tile([1, 1], F32)
            nc.vector.reciprocal(ru[:], u[:])
            iou = sb.tile([1, 1], F32)
            _tt(nc, iou[:], dI[:], ru[:], A.mult)
            r2 = sb.tile([1, 1], F32)
            nc.vector.reciprocal(r2[:], psS[0:1, 1:2])
            cl = sb.tile([1, 1], F32)
            _tt(nc, cl[:], psS[0:1, 0:1], r2[:], A.mult)
            tmp2 = sb.tile([1, 1], F32)
            nc.vector.scalar_tensor_tensor(
                tmp2[:], iou[:], -1.0, cl[:], A.mult, A.add
            )
            lossv = sb.tile([1, 1], F32)
            nc.vector.tensor_scalar(lossv[:], tmp2[:], 1.0, None, A.add)
            nc.sync.dma_start(loss[:], lossv[:])

    nc.compile()
    return nc


def _constants():
    ones1 = np.ones((1, 1), dtype=np.float32)
    ones32 = np.ones((B, 1), dtype=np.float32)
    emat = np.zeros((B, P), dtype=np.float32)
    for k in range(B):
        emat[k, k * SEG : (k + 1) * SEG] = 1.0
    ematt = np.ascontiguousarray(emat.T)
    sel4 = (np.arange(P)[:, None] % SEG == np.arange(SEG)[None, :]).astype(np.float32)
    resetv = np.zeros((1, P), dtype=np.float32)
    resetv[0, ::SEG] = NEGBIG
    id128 = np.eye(P, dtype=np.float32)
    iotag = (
        float(FREE) * np.arange(P, dtype=np.float32)[:, None]
        + np.arange(FREE, dtype=np.float32)[None, :]
        - GOFF
    ).astype(np.float32)
    # global row base in (idx - 65536) space relative to the (idx - 4096) space
    rb32 = np.repeat(
        (2048.0 * np.arange(B, dtype=np.float32) - (GOFF - OFF))[:, None], 2, axis=1
    ).astype(np.float32)
    pk1 = np.concatenate([sel4, ematt], axis=1).astype(np.float32)
    pk3 = np.concatenate([resetv, ones1], axis=1).astype(np.float32)
    return {"pk1": pk1, "pk3": pk3, "emat": emat, "ones32": ones32, "rb32": rb32,
            "id128": id128, "iotag": iotag}


def build_in_maps(c_ids, gt_start_positions, gt_end_positions, start_logits, end_logits):
    consts = _constants()
    cidf = np.ascontiguousarray(np.asarray(c_ids), dtype=np.int16).reshape(P, FREE)
    slf = np.ascontiguousarray(np.asarray(start_logits), dtype=np.float32).reshape(P, FREE)
    elf = np.ascontiguousarray(np.asarray(end_logits), dtype=np.float32).reshape(P, FREE)
    gt32 = np.stack(
        [
            np.asarray(gt_start_positions).astype(np.float32) - np.float32(OFF),
            np.asarray(gt_end_positions).astype(np.float32) - np.float32(OFF),
        ],
        axis=1,
    ).astype(np.float32)
    pk2 = np.concatenate(
        [consts["emat"], consts["ones32"], gt32, consts["rb32"]], axis=1
    ).astype(np.float32)
    core_map = {
        "sl": slf, "el": elf, "cid": cidf,
        "pk1": consts["pk1"], "pk2": pk2, "pk3": consts["pk3"],
        "id128": consts["id128"], "iotag": consts["iotag"],
    }
    return [dict(core_map) for _ in range(NCORES)]


def kernel(c_ids, gt_start_positions, gt_end_positions, start_logits, end_logits):
    if "nc" not in _CACHE:
        _CACHE["nc"] = _build_program()
    nc = _CACHE["nc"]
    in_maps = build_in_maps(
        c_ids, gt_start_positions, gt_end_positions, start_logits, end_logits
    )
    res = run_bass_kernel_spmd(nc, in_maps, core_ids=list(range(NCORES)))
    return np.asarray(res.results[0]["loss"], dtype=np.float32).reshape(())
